# revision 44
# baseline (speedup 1.0000x reference)
"""ArcFace loss kernel for Trainium2, SPMD over 8 NeuronCores.

Reference (N=512 batch, D=512 dim, C=100000 classes, S=1):
    w_n   = w / ||w||_D
    cos   = emb @ w_n                  # emb rows are unit-norm
    logit = cos(arccos(cos) + target*0.5) * 64
    out   = softmax(logit, axis=0)     # over the BATCH axis

Sharding: classes split across 8 cores (tensor parallel). The axis-0
softmax reduces over batch, which is the on-core free axis, so there are
no collectives.

Key design points:
  * Matmuls in fp16 (host casts w/emb): 1 cycle/row on TensorE vs 4 for
    fp32, FWL-eligible weight loads (fp32/f32r LDWEIGHTS serializes with
    the matmuls), and half the input HBM traffic. ~11-bit mantissa keeps
    rel err ~2e-3.
  * Margin handled SPARSELY: bulk path is exp(SCALE*cos); a tiny side
    pipeline computes corrected values for the N=512 one-hot targets
    from host-gathered target columns, fixes the softmax denominators
    via one-hot matmuls (dSM), and emits the corrected output values as
    a tiny `patch` tensor that the host places (indexing only).
  * exp runs on ScalarE straight from PSUM with the per-class norm as
    the activation scale; accum_out yields the softmax denominator in
    the same pass (no separate reduce).
  * All rsqrt/sqrt are Newton iterations on VectorE (norms live in a
    narrow range, so a constant seed converges in 4 steps). ScalarE uses
    only Exp/Square -> a single ACT table set, no table-switch thrash.
  * Norm computation is software-pipelined 2 groups ahead of the main
    matmul stream so the in-order PE queue never waits on it.
  * Output is written bf16 and upcast on host (halves output traffic).
  * Measured: ~175-200us on silicon, rel l2 err 2.4e-3 (8 cores).
"""

import math
import os
import sys

for _p in ("/opt/trn_rl_repo", "/root/.axon_site/_ro/trn_rl_repo"):
    if os.path.isdir(_p) and _p not in sys.path:
        sys.path.append(_p)

import numpy as np

import concourse.bass as bass
import concourse.tile as tile
from concourse import bacc, mybir
from concourse.bass_utils import run_bass_kernel_spmd

N = 512
D = 512
C = 100000
N_CORES = 8
C_SHARD = C // N_CORES          # 12500
C_PAD = 12800                   # 100 tiles of 128
N_TILES = C_PAD // 128          # 100
MARGIN = 0.5
SCALE = 64.0
COS_M = math.cos(MARGIN)
SIN_M = math.sin(MARGIN)
SS = (SCALE * SIN_M) ** 2

KCHUNKS = D // 128              # 4
GROUP_COLS = 1280               # 10 class-tiles per group
N_GROUPS = C_PAD // GROUP_COLS  # 10
TILES_PER_GROUP = GROUP_COLS // 128     # 10

F32 = mybir.dt.float32
F32R = mybir.dt.float32r
F16 = mybir.dt.float16
BF16 = mybir.dt.bfloat16
I32 = mybir.dt.int32
AFT = mybir.ActivationFunctionType
ALU = mybir.AluOpType


def _newton_rsqrt(nc, pool, x_ap, shape, seed, iters, tag):
    """r ~= 1/sqrt(x) via Newton on VectorE: r <- r*(1.5 - 0.5*x*r^2).

    The seed is produced as x*0 + seed (not memset) so the op depends on
    x -- keeps the scheduler from hoisting all seeds to t=0 and
    deadlocking on pool slots."""
    r = pool.tile(shape, F32, tag=f"{tag}_r")
    nc.vector.tensor_scalar(r[:], x_ap, 0.0, seed, op0=ALU.mult,
                            op1=ALU.add)
    for i in range(iters):
        r2 = pool.tile(shape, F32, tag=f"{tag}_r2")
        nc.vector.tensor_tensor(r2[:], r[:], r[:], op=ALU.mult)
        t = pool.tile(shape, F32, tag=f"{tag}_t")
        nc.vector.tensor_tensor(t[:], x_ap, r2[:], op=ALU.mult)
        u = pool.tile(shape, F32, tag=f"{tag}_u")
        nc.vector.tensor_scalar(u[:], t[:], -0.5, 1.5, op0=ALU.mult,
                                op1=ALU.add)
        rn = pool.tile(shape, F32, tag=f"{tag}_rn")
        nc.vector.tensor_tensor(rn[:], r[:], u[:], op=ALU.mult)
        r = rn
    return r


def build_program():
    nc = bacc.Bacc("TRN2", target_bir_lowering=False, debug=False,
                   num_devices=N_CORES)

    embT = nc.dram_tensor("embT", [D, N], F16, kind="ExternalInput").ap()
    w = nc.dram_tensor("w", [D, C_PAD], F16, kind="ExternalInput").ap()
    wt = nc.dram_tensor("wt", [D, N], F16, kind="ExternalInput").ap()
    h1 = nc.dram_tensor("h1", [N, 128], F32, kind="ExternalInput").ap()
    h1t = nc.dram_tensor("h1t", [128, N], F32, kind="ExternalInput").ap()
    h2j = nc.dram_tensor("h2j", [N, N_TILES], F32, kind="ExternalInput").ap()
    out = nc.dram_tensor("out", [C_PAD, N], BF16, kind="ExternalOutput").ap()
    patch = nc.dram_tensor("patch", [128, KCHUNKS], F32,
                           kind="ExternalOutput").ap()

    nscr = nc.dram_tensor("nscratch", [1, C_PAD], F32).ap()
    zscr = nc.dram_tensor("zscratch", [1, N], F32).ap()
    tscr = nc.dram_tensor("tscratch", [1, N], F32).ap()

    w_ck = w.rearrange("(c p) m -> p c m", p=128)        # [128, 4, C_PAD]
    embT_ck = embT.rearrange("(c p) n -> p c n", p=128)  # [128, 4, N]
    wt_ck = wt.rearrange("(c p) n -> p c n", p=128)      # [128, 4, N]
    out_t = out.rearrange("(t p) n -> p t n", p=128)     # [128, 100, N]

    from contextlib import ExitStack

    with tile.TileContext(nc) as tc, ExitStack() as ctx:
        consts = ctx.enter_context(tc.tile_pool(name="consts", bufs=1))
        persist = ctx.enter_context(tc.tile_pool(name="persist", bufs=1))
        wpool = ctx.enter_context(tc.tile_pool(name="w", bufs=5))
        wsqpool = ctx.enter_context(tc.tile_pool(name="wsq", bufs=1))
        npool = ctx.enter_context(tc.tile_pool(name="norms", bufs=4))
        r64pool = ctx.enter_context(tc.tile_pool(name="r64", bufs=4))
        epool = ctx.enter_context(tc.tile_pool(name="ew", bufs=6))
        opool = ctx.enter_context(tc.tile_pool(name="o", bufs=4))
        spool = ctx.enter_context(tc.tile_pool(name="small", bufs=4))
        sidep = ctx.enter_context(tc.tile_pool(name="side", bufs=1))
        newtp = ctx.enter_context(tc.tile_pool(name="newt", bufs=2))
        zpool = ctx.enter_context(tc.tile_pool(name="z", bufs=4,
                                               space="PSUM"))
        nqpool = ctx.enter_context(tc.tile_pool(name="nq", bufs=2,
                                                space="PSUM"))
        gpool = ctx.enter_context(tc.tile_pool(name="g", bufs=2,
                                               space="PSUM"))

        # ---------------- constants & persistent tiles ----------------
        et = consts.tile([128, KCHUNKS * N], F16)
        nc.sync.dma_start(et[:], embT_ck[:, :, :])

        ones_h = consts.tile([128, 1], F16)
        nc.vector.memset(ones_h[:], 1.0)
        ones32 = consts.tile([128, 1], F32)
        nc.vector.memset(ones32[:], 1.0)

        dsm_all = persist.tile([128, N_TILES], F32)
        rp_all = persist.tile([128, N_TILES], F32)

        # ---------------- pipelined groups (first load hoisted) --------
        wg_of = {}
        out_dmas = []

        def load_and_square(g):
            g0 = g * GROUP_COLS
            wg = []
            wsqs = []
            for c in range(KCHUNKS):
                t = wpool.tile([128, GROUP_COLS], F16, tag=f"wg{c}")
                nc.sync.dma_start(
                    t[:], w_ck[:, c, g0:g0 + GROUP_COLS])
                wg.append(t)
            wg_of[g] = (wg, wsqs)

        load_and_square(0)

        # ---------------- side pipeline: margin corrections ------------
        wtt = sidep.tile([128, KCHUNKS * N], F16)
        nc.sync.dma_start(wtt[:], wt_ck[:, :, :])

        h1c = sidep.tile([128, KCHUNKS * 128], F32)
        nc.sync.dma_start(
            h1c[:], h1.rearrange("(c p) m -> p c m", p=128)[:, :, :])
        h1tc = sidep.tile([128, N], F32)
        nc.sync.dma_start(h1tc[:], h1t[:, :])
        h2c = sidep.tile([128, KCHUNKS * N_TILES], F32)
        nc.sync.dma_start(
            h2c[:], h2j.rearrange("(c p) m -> p c m", p=128)[:, :, :])
        # z_t[j] = emb[j] . w[:, label_j]
        p4 = sidep.tile([128, KCHUNKS * N], F32)
        nc.vector.tensor_tensor(p4[:], et[:], wtt[:], op=ALU.mult)
        zq = nqpool.tile([1, N], F32, tag="nq")
        for c in range(KCHUNKS):
            nc.tensor.matmul(zq[:], ones32[:], p4[:, c * N:(c + 1) * N],
                             start=(c == 0), stop=(c == KCHUNKS - 1))
        zrow = sidep.tile([1, N], F32)
        nc.vector.tensor_copy(zrow[:], zq[:])
        nc.sync.dma_start(zscr[:], zrow[:])
        zcol = sidep.tile([128, KCHUNKS], F32)
        nc.sync.dma_start(
            zcol[:], zscr.rearrange("a (c p) -> p (a c)", p=128))

        # target-class squared norms
        wsq_t = sidep.tile([128, KCHUNKS * N], F32)
        nc.scalar.activation(wsq_t[:], wtt[:], AFT.Square)
        nq_t = nqpool.tile([1, N], F32, tag="nq")
        for c in range(KCHUNKS):
            nc.tensor.matmul(nq_t[:], ones32[:], wsq_t[:, c * N:(c + 1) * N],
                             start=(c == 0), stop=(c == KCHUNKS - 1))
        ntrow = sidep.tile([1, N], F32)
        nc.vector.tensor_copy(ntrow[:], nq_t[:])
        nc.sync.dma_start(tscr[:], ntrow[:])
        nst = sidep.tile([128, KCHUNKS], F32)
        nc.sync.dma_start(
            nst[:], tscr.rearrange("a (c p) -> p (a c)", p=128))

        # r64t = 64/sqrt(nst): Newton rsqrt (nsq in ~[320, 730])
        rt = _newton_rsqrt(nc, newtp, nst[:], [128, KCHUNKS],
                           0.0447, 4, "rt")
        r64t = sidep.tile([128, KCHUNKS], F32)
        nc.vector.tensor_scalar(r64t[:], rt[:], SCALE, None, op0=ALU.mult)
        cos64 = sidep.tile([128, KCHUNKS], F32)
        nc.vector.tensor_tensor(cos64[:], zcol[:], r64t[:], op=ALU.mult)
        # sin term: 30.683*sin(theta) = sqrt(su), su = SS - SS*cos^2
        s2t = sidep.tile([128, KCHUNKS], F32)
        nc.scalar.activation(s2t[:], cos64[:], AFT.Square, scale=1.0 / SCALE)
        su = sidep.tile([128, KCHUNKS], F32)
        nc.vector.tensor_scalar(su[:], s2t[:], -SS, SS, op0=ALU.mult,
                                op1=ALU.add)
        rsu = _newton_rsqrt(nc, newtp, su[:], [128, KCHUNKS],
                            0.0333, 4, "rsu")
        dmt = sidep.tile([128, KCHUNKS], F32)
        nc.vector.tensor_tensor(dmt[:], su[:], rsu[:], op=ALU.mult)
        m1t = sidep.tile([128, KCHUNKS], F32)
        nc.vector.tensor_scalar(m1t[:], cos64[:], COS_M, None, op0=ALU.mult)
        lgm = sidep.tile([128, KCHUNKS], F32)
        nc.vector.tensor_tensor(lgm[:], m1t[:], dmt[:], op=ALU.subtract)
        en = sidep.tile([128, KCHUNKS], F32)
        nc.scalar.activation(en[:], lgm[:], AFT.Exp)
        eold = sidep.tile([128, KCHUNKS], F32)
        nc.scalar.activation(eold[:], cos64[:], AFT.Exp)
        dcol = sidep.tile([128, KCHUNKS], F32)
        nc.vector.tensor_tensor(dcol[:], en[:], eold[:], op=ALU.subtract)

        # dSM[p, t] = sum_j H1[j,p] * H2J[j,t] * d[j]
        dq = gpool.tile([128, N_TILES], F32, tag="g")
        for c in range(KCHUNKS):
            rhs = sidep.tile([128, N_TILES], F32, tag=f"dr{c}")
            nc.vector.tensor_scalar(rhs[:],
                                    h2c[:, c * N_TILES:(c + 1) * N_TILES],
                                    dcol[:, c:c + 1], None, op0=ALU.mult)
            nc.tensor.matmul(dq[:], h1c[:, c * 128:(c + 1) * 128], rhs[:],
                             start=(c == 0), stop=(c == KCHUNKS - 1))
        nc.vector.tensor_copy(dsm_all[:], dq[:])

        # ---------------- pipelined groups ------------------------------
        def norm_chain(g):
            g0 = g * GROUP_COLS
            t0 = g * TILES_PER_GROUP
            wg, _ = wg_of[g]
            wsqs = []
            for c in range(KCHUNKS):
                sq = wsqpool.tile([128, GROUP_COLS], F16, tag=f"wsq{c}")
                nc.vector.tensor_tensor(sq[:], wg[c][:], wg[c][:],
                                        op=ALU.mult)
                wsqs.append(sq)
            # pre-sum the 4 chunks on DVE so the partition-reduce needs one
            # ones-matmul per strip instead of four
            sa = wsqpool.tile([128, GROUP_COLS], F16, tag="wsqa")
            nc.vector.tensor_tensor(sa[:], wsqs[0][:], wsqs[1][:],
                                    op=ALU.add)
            sb = wsqpool.tile([128, GROUP_COLS], F16, tag="wsqb")
            nc.vector.tensor_tensor(sb[:], wsqs[2][:], wsqs[3][:],
                                    op=ALU.add)
            st = wsqpool.tile([128, GROUP_COLS], F16, tag="wsqt")
            nc.vector.tensor_tensor(st[:], sa[:], sb[:], op=ALU.add)
            nrow = npool.tile([1, GROUP_COLS], F32, tag="nrow")
            s0 = 0
            for sw in (512, 512, 256):
                nq = nqpool.tile([1, 512], F32, tag="nq")
                nc.tensor.matmul(nq[:1, :sw], ones_h[:], st[:, s0:s0 + sw],
                                 start=True, stop=True)
                nc.scalar.copy(nrow[:, s0:s0 + sw], nq[:1, :sw])
                s0 += sw
            nc.sync.dma_start(nscr[:, g0:g0 + GROUP_COLS], nrow[:])
            ncol = npool.tile([128, TILES_PER_GROUP], F32, tag="ncol")
            nc.sync.dma_start(
                ncol[:],
                nscr[:, g0:g0 + GROUP_COLS].rearrange(
                    "a (t p) -> p (a t)", p=128))
            rg = _newton_rsqrt(nc, newtp, ncol[:], [128, TILES_PER_GROUP],
                               0.0447, 4, "nr")
            r64g = r64pool.tile([128, TILES_PER_GROUP], F32, tag="r64")
            nc.vector.tensor_scalar(r64g[:], rg[:], SCALE, None, op0=ALU.mult)
            wg_of[g] = (*wg_of[g], r64g)

        def main_group(g):
            wg, _, r64g = wg_of[g]
            t0 = g * TILES_PER_GROUP
            obuf = None
            ostart = 0
            for m in range(TILES_PER_GROUP):
                ti = t0 + m
                z = zpool.tile([128, N], F32, tag="z")
                for c in range(KCHUNKS):
                    nc.tensor.matmul(
                        z[:], wg[c][:, m * 128:(m + 1) * 128],
                        et[:, c * N:(c + 1) * N],
                        start=(c == 0), stop=(c == KCHUNKS - 1))
                ex = epool.tile([128, N], BF16, tag="ex")
                sm = spool.tile([128, 1], F32, tag="sm")
                nc.scalar.activation(ex[:], z[:], AFT.Exp,
                                     scale=r64g[:, m:m + 1],
                                     accum_out=sm[:])
                smf = spool.tile([128, 1], F32, tag="smf")
                nc.vector.tensor_tensor(smf[:], sm[:],
                                        dsm_all[:, ti:ti + 1], op=ALU.add)
                nc.vector.reciprocal(rp_all[:, ti:ti + 1], smf[:])
                o = opool.tile([128, N], BF16, tag="o")
                nc.vector.tensor_scalar(o[:], ex[:], rp_all[:, ti:ti + 1],
                                        None, op0=ALU.mult)
                out_dmas.append(
                    nc.sync.dma_start(out_t[:, ti, :], o[:]))
            del wg_of[g]

        for g in range(1, 3):
            load_and_square(g)
        norm_chain(0)
        norm_chain(1)
        for g in range(N_GROUPS):
            main_group(g)
            if g + 3 < N_GROUPS:
                load_and_square(g + 3)
            if g + 2 < N_GROUPS:
                norm_chain(g + 2)

        # ---------------- tail: patch target entries --------------------
        v_all = spool.tile([128, KCHUNKS], F32, tag="vall")
        for c in range(KCHUNKS):
            gq = gpool.tile([128, N_TILES], F32, tag="g")
            nc.tensor.matmul(gq[:], h1tc[:, c * 128:(c + 1) * 128],
                             rp_all[:], start=True, stop=True)
            g2 = spool.tile([128, N_TILES], F32, tag="g2")
            nc.vector.tensor_tensor(g2[:], gq[:],
                                    h2c[:, c * N_TILES:(c + 1) * N_TILES],
                                    op=ALU.mult)
            rpt = spool.tile([128, 1], F32, tag="rpt")
            nc.vector.reduce_sum(rpt[:], g2[:], axis=mybir.AxisListType.X)
            nc.vector.tensor_tensor(v_all[:, c:c + 1], en[:, c:c + 1],
                                    rpt[:], op=ALU.mult)
        nc.sync.dma_start(patch[:, :], v_all[:])

    nc.compile()
    return nc


_NC_CACHE = None


def _get_program():
    global _NC_CACHE
    if _NC_CACHE is None:
        _NC_CACHE = build_program()
    return _NC_CACHE


def _shard_inputs(embedding_batch, w_param, target_batch):
    emb = np.ascontiguousarray(embedding_batch, dtype=np.float32)
    wp = np.asarray(w_param, dtype=np.float32).reshape(D, C)
    tgt = np.asarray(target_batch, dtype=np.float32)

    embT = np.ascontiguousarray(emb.T.astype(np.float16))
    labels = np.argmax(tgt, axis=1).astype(np.int64)
    wt = np.ascontiguousarray(wp[:, labels].astype(np.float16))

    js = np.arange(N)
    in_maps = []
    for k in range(N_CORES):
        lo = k * C_SHARD
        in_shard = (labels >= lo) & (labels < lo + C_SHARD)
        lc = np.where(in_shard, labels - lo, 0)

        wk = np.ones((D, C_PAD), dtype=np.float16)
        wk[:, :C_SHARD] = wp[:, lo:lo + C_SHARD].astype(np.float16)

        h1 = np.zeros((N, 128), dtype=np.float32)
        h1[js[in_shard], lc[in_shard] % 128] = 1.0
        h2 = np.zeros((N, N_TILES), dtype=np.float32)
        h2[js[in_shard], lc[in_shard] // 128] = 1.0
        in_maps.append({
            "embT": embT, "w": wk, "wt": wt,
            "h1": h1, "h1t": np.ascontiguousarray(h1.T),
            "h2j": h2,
        })
    return in_maps


def run(inputs, trace=False):
    nc = _get_program()
    in_maps = _shard_inputs(**inputs)
    res = run_bass_kernel_spmd(nc, in_maps, core_ids=list(range(N_CORES)),
                               trace=trace)
    full = np.empty((N, C), dtype=np.float32)
    for k in range(N_CORES):
        full[:, k * C_SHARD:(k + 1) * C_SHARD] = \
            res.results[k]["out"][:C_SHARD, :].astype(np.float32).T
    # place the device-computed margin patch values at the target entries
    labels = np.argmax(np.asarray(inputs["target_batch"]), axis=1)
    js = np.arange(N)
    owner = labels // C_SHARD
    for k in range(N_CORES):
        sel = owner == k
        pk = np.asarray(res.results[k]["patch"], dtype=np.float32)
        full[js[sel], labels[sel]] = pk[js[sel] % 128, js[sel] // 128]
    return full, res


def kernel(embedding_batch, w_param, target_batch):
    full, _ = run(dict(embedding_batch=embedding_batch, w_param=w_param,
                       target_batch=target_batch))
    return full


# revision 45
# speedup vs baseline: 1.1941x; 1.1941x over previous
"""ArcFace loss kernel for Trainium2, SPMD over 8 NeuronCores.

Reference (N=512 batch, D=512 dim, C=100000 classes, S=1):
    w_n   = w / ||w||_D
    cos   = emb @ w_n                  # emb rows are unit-norm
    logit = cos(arccos(cos) + target*0.5) * 64
    out   = softmax(logit, axis=0)     # over the BATCH axis

Sharding: classes split across 8 cores (tensor parallel). The axis-0
softmax reduces over batch, which is the on-core free axis, so there are
no collectives.

Key design points:
  * Matmuls in fp16 (host casts w/emb): 1 cycle/row on TensorE vs 4 for
    fp32, FWL-eligible weight loads (fp32/f32r LDWEIGHTS serializes with
    the matmuls), and half the input HBM traffic. ~11-bit mantissa keeps
    rel err ~2e-3.
  * Margin handled SPARSELY: bulk path is exp(SCALE*cos); a tiny side
    pipeline computes corrected values for the N=512 one-hot targets
    from host-gathered target columns, fixes the softmax denominators
    via one-hot matmuls (dSM), and emits the corrected output values as
    a tiny `patch` tensor that the host places (indexing only).
  * exp runs on ScalarE straight from PSUM with the per-class norm as
    the activation scale; accum_out yields the softmax denominator in
    the same pass (no separate reduce).
  * All rsqrt/sqrt are Newton iterations on VectorE (norms live in a
    narrow range, so a constant seed converges in 4 steps). ScalarE uses
    only Exp/Square -> a single ACT table set, no table-switch thrash.
  * Norm computation is software-pipelined 2 groups ahead of the main
    matmul stream so the in-order PE queue never waits on it.
  * Output is written bf16 and upcast on host (halves output traffic).
  * Measured: ~175-200us on silicon, rel l2 err 2.4e-3 (8 cores).
"""

import math
import os
import sys

for _p in ("/opt/trn_rl_repo", "/root/.axon_site/_ro/trn_rl_repo"):
    if os.path.isdir(_p) and _p not in sys.path:
        sys.path.append(_p)

import numpy as np

import concourse.bass as bass
import concourse.tile as tile
from concourse import bacc, mybir
from concourse.bass_utils import run_bass_kernel_spmd

N = 512
D = 512
C = 100000
N_CORES = 8
C_SHARD = C // N_CORES          # 12500
C_PAD = 12800                   # 100 tiles of 128
N_TILES = C_PAD // 128          # 100
MARGIN = 0.5
SCALE = 64.0
COS_M = math.cos(MARGIN)
SIN_M = math.sin(MARGIN)
SS = (SCALE * SIN_M) ** 2

KCHUNKS = D // 128              # 4
GROUP_COLS = 1280               # 10 class-tiles per group
N_GROUPS = C_PAD // GROUP_COLS  # 10
TILES_PER_GROUP = GROUP_COLS // 128     # 10

F32 = mybir.dt.float32
F32R = mybir.dt.float32r
F16 = mybir.dt.float16
BF16 = mybir.dt.bfloat16
I32 = mybir.dt.int32
AFT = mybir.ActivationFunctionType
ALU = mybir.AluOpType


def _newton_rsqrt(nc, pool, x_ap, shape, seed, iters, tag):
    """r ~= 1/sqrt(x) via Newton on VectorE: r <- r*(1.5 - 0.5*x*r^2).

    The seed is produced as x*0 + seed (not memset) so the op depends on
    x -- keeps the scheduler from hoisting all seeds to t=0 and
    deadlocking on pool slots."""
    r = pool.tile(shape, F32, tag=f"{tag}_r")
    nc.vector.tensor_scalar(r[:], x_ap, 0.0, seed, op0=ALU.mult,
                            op1=ALU.add)
    for i in range(iters):
        r2 = pool.tile(shape, F32, tag=f"{tag}_r2")
        nc.vector.tensor_tensor(r2[:], r[:], r[:], op=ALU.mult)
        t = pool.tile(shape, F32, tag=f"{tag}_t")
        nc.vector.tensor_tensor(t[:], x_ap, r2[:], op=ALU.mult)
        u = pool.tile(shape, F32, tag=f"{tag}_u")
        nc.vector.tensor_scalar(u[:], t[:], -0.5, 1.5, op0=ALU.mult,
                                op1=ALU.add)
        rn = pool.tile(shape, F32, tag=f"{tag}_rn")
        nc.vector.tensor_tensor(rn[:], r[:], u[:], op=ALU.mult)
        r = rn
    return r


def build_program():
    nc = bacc.Bacc("TRN2", target_bir_lowering=False, debug=False,
                   num_devices=N_CORES)

    embT = nc.dram_tensor("embT", [D, N], F16, kind="ExternalInput").ap()
    w = nc.dram_tensor("w", [N_GROUPS, KCHUNKS, 128, GROUP_COLS],
                       F16, kind="ExternalInput").ap()
    wt = nc.dram_tensor("wt", [D, N], F16, kind="ExternalInput").ap()
    h1 = nc.dram_tensor("h1", [N, 128], F32, kind="ExternalInput").ap()
    h1t = nc.dram_tensor("h1t", [128, N], F32, kind="ExternalInput").ap()
    h2j = nc.dram_tensor("h2j", [N, N_TILES], F32, kind="ExternalInput").ap()
    out = nc.dram_tensor("out", [C_PAD, N], BF16, kind="ExternalOutput").ap()
    patch = nc.dram_tensor("patch", [128, KCHUNKS], F32,
                           kind="ExternalOutput").ap()

    nscr = nc.dram_tensor("nscratch", [1, C_PAD], F32).ap()
    zscr = nc.dram_tensor("zscratch", [1, N], F32).ap()
    tscr = nc.dram_tensor("tscratch", [1, N], F32).ap()

    embT_ck = embT.rearrange("(c p) n -> p c n", p=128)  # [128, 4, N]
    wt_ck = wt.rearrange("(c p) n -> p c n", p=128)      # [128, 4, N]
    out_t = out.rearrange("(t p) n -> p t n", p=128)     # [128, 100, N]

    from contextlib import ExitStack

    with tile.TileContext(nc) as tc, ExitStack() as ctx:
        consts = ctx.enter_context(tc.tile_pool(name="consts", bufs=1))
        persist = ctx.enter_context(tc.tile_pool(name="persist", bufs=1))
        wpool = ctx.enter_context(tc.tile_pool(name="w", bufs=5))
        wsqpool = ctx.enter_context(tc.tile_pool(name="wsq", bufs=1))
        npool = ctx.enter_context(tc.tile_pool(name="norms", bufs=4))
        r64pool = ctx.enter_context(tc.tile_pool(name="r64", bufs=4))
        epool = ctx.enter_context(tc.tile_pool(name="ew", bufs=6))
        opool = ctx.enter_context(tc.tile_pool(name="o", bufs=4))
        spool = ctx.enter_context(tc.tile_pool(name="small", bufs=4))
        sidep = ctx.enter_context(tc.tile_pool(name="side", bufs=1))
        newtp = ctx.enter_context(tc.tile_pool(name="newt", bufs=2))
        zpool = ctx.enter_context(tc.tile_pool(name="z", bufs=4,
                                               space="PSUM"))
        nqpool = ctx.enter_context(tc.tile_pool(name="nq", bufs=2,
                                                space="PSUM"))
        gpool = ctx.enter_context(tc.tile_pool(name="g", bufs=2,
                                               space="PSUM"))

        # ---------------- constants & persistent tiles ----------------
        et = consts.tile([128, KCHUNKS * N], F16)
        nc.sync.dma_start(et[:], embT_ck[:, :, :])

        ones_h = consts.tile([128, 1], F16)
        nc.vector.memset(ones_h[:], 1.0)
        ones32 = consts.tile([128, 1], F32)
        nc.vector.memset(ones32[:], 1.0)

        dsm_all = persist.tile([128, N_TILES], F32)
        rp_all = persist.tile([128, N_TILES], F32)

        # ---------------- pipelined groups (first load hoisted) --------
        wg_of = {}
        out_dmas = []

        def load_and_square(g):
            g0 = g * GROUP_COLS
            wg = []
            wsqs = []
            for c in range(KCHUNKS):
                t = wpool.tile([128, GROUP_COLS], F16, tag=f"wg{c}")
                nc.sync.dma_start(t[:], w[g, c, :, :])
                wg.append(t)
            wg_of[g] = (wg, wsqs)

        load_and_square(0)

        # ---------------- side pipeline: margin corrections ------------
        wtt = sidep.tile([128, KCHUNKS * N], F16)
        nc.sync.dma_start(wtt[:], wt_ck[:, :, :])

        h1c = sidep.tile([128, KCHUNKS * 128], F32)
        nc.sync.dma_start(
            h1c[:], h1.rearrange("(c p) m -> p c m", p=128)[:, :, :])
        h1tc = sidep.tile([128, N], F32)
        nc.sync.dma_start(h1tc[:], h1t[:, :])
        h2c = sidep.tile([128, KCHUNKS * N_TILES], F32)
        nc.sync.dma_start(
            h2c[:], h2j.rearrange("(c p) m -> p c m", p=128)[:, :, :])
        # z_t[j] = emb[j] . w[:, label_j]
        p4 = sidep.tile([128, KCHUNKS * N], F32)
        nc.vector.tensor_tensor(p4[:], et[:], wtt[:], op=ALU.mult)
        zq = nqpool.tile([1, N], F32, tag="nq")
        for c in range(KCHUNKS):
            nc.tensor.matmul(zq[:], ones32[:], p4[:, c * N:(c + 1) * N],
                             start=(c == 0), stop=(c == KCHUNKS - 1))
        zrow = sidep.tile([1, N], F32)
        nc.vector.tensor_copy(zrow[:], zq[:])
        nc.sync.dma_start(zscr[:], zrow[:])
        zcol = sidep.tile([128, KCHUNKS], F32)
        nc.sync.dma_start(
            zcol[:], zscr.rearrange("a (c p) -> p (a c)", p=128))

        # target-class squared norms
        wsq_t = sidep.tile([128, KCHUNKS * N], F32)
        nc.scalar.activation(wsq_t[:], wtt[:], AFT.Square)
        nq_t = nqpool.tile([1, N], F32, tag="nq")
        for c in range(KCHUNKS):
            nc.tensor.matmul(nq_t[:], ones32[:], wsq_t[:, c * N:(c + 1) * N],
                             start=(c == 0), stop=(c == KCHUNKS - 1))
        ntrow = sidep.tile([1, N], F32)
        nc.vector.tensor_copy(ntrow[:], nq_t[:])
        nc.sync.dma_start(tscr[:], ntrow[:])
        nst = sidep.tile([128, KCHUNKS], F32)
        nc.sync.dma_start(
            nst[:], tscr.rearrange("a (c p) -> p (a c)", p=128))

        # r64t = 64/sqrt(nst): Newton rsqrt (nsq in ~[320, 730])
        rt = _newton_rsqrt(nc, newtp, nst[:], [128, KCHUNKS],
                           0.0447, 4, "rt")
        r64t = sidep.tile([128, KCHUNKS], F32)
        nc.vector.tensor_scalar(r64t[:], rt[:], SCALE, None, op0=ALU.mult)
        cos64 = sidep.tile([128, KCHUNKS], F32)
        nc.vector.tensor_tensor(cos64[:], zcol[:], r64t[:], op=ALU.mult)
        # sin term: 30.683*sin(theta) = sqrt(su), su = SS - SS*cos^2
        s2t = sidep.tile([128, KCHUNKS], F32)
        nc.scalar.activation(s2t[:], cos64[:], AFT.Square, scale=1.0 / SCALE)
        su = sidep.tile([128, KCHUNKS], F32)
        nc.vector.tensor_scalar(su[:], s2t[:], -SS, SS, op0=ALU.mult,
                                op1=ALU.add)
        rsu = _newton_rsqrt(nc, newtp, su[:], [128, KCHUNKS],
                            0.0333, 4, "rsu")
        dmt = sidep.tile([128, KCHUNKS], F32)
        nc.vector.tensor_tensor(dmt[:], su[:], rsu[:], op=ALU.mult)
        m1t = sidep.tile([128, KCHUNKS], F32)
        nc.vector.tensor_scalar(m1t[:], cos64[:], COS_M, None, op0=ALU.mult)
        lgm = sidep.tile([128, KCHUNKS], F32)
        nc.vector.tensor_tensor(lgm[:], m1t[:], dmt[:], op=ALU.subtract)
        en = sidep.tile([128, KCHUNKS], F32)
        nc.scalar.activation(en[:], lgm[:], AFT.Exp)
        eold = sidep.tile([128, KCHUNKS], F32)
        nc.scalar.activation(eold[:], cos64[:], AFT.Exp)
        dcol = sidep.tile([128, KCHUNKS], F32)
        nc.vector.tensor_tensor(dcol[:], en[:], eold[:], op=ALU.subtract)

        # dSM[p, t] = sum_j H1[j,p] * H2J[j,t] * d[j]
        dq = gpool.tile([128, N_TILES], F32, tag="g")
        for c in range(KCHUNKS):
            rhs = sidep.tile([128, N_TILES], F32, tag=f"dr{c}")
            nc.vector.tensor_scalar(rhs[:],
                                    h2c[:, c * N_TILES:(c + 1) * N_TILES],
                                    dcol[:, c:c + 1], None, op0=ALU.mult)
            nc.tensor.matmul(dq[:], h1c[:, c * 128:(c + 1) * 128], rhs[:],
                             start=(c == 0), stop=(c == KCHUNKS - 1))
        nc.vector.tensor_copy(dsm_all[:], dq[:])

        # ---------------- pipelined groups ------------------------------
        def norm_chain(g):
            g0 = g * GROUP_COLS
            t0 = g * TILES_PER_GROUP
            wg, _ = wg_of[g]
            wsqs = []
            for c in range(KCHUNKS):
                sq = wsqpool.tile([128, GROUP_COLS], F16, tag=f"wsq{c}")
                nc.vector.tensor_tensor(sq[:], wg[c][:], wg[c][:],
                                        op=ALU.mult)
                wsqs.append(sq)
            # pre-sum the 4 chunks on DVE so the partition-reduce needs one
            # ones-matmul per strip instead of four
            sa = wsqpool.tile([128, GROUP_COLS], F16, tag="wsqa")
            nc.vector.tensor_tensor(sa[:], wsqs[0][:], wsqs[1][:],
                                    op=ALU.add)
            sb = wsqpool.tile([128, GROUP_COLS], F16, tag="wsqb")
            nc.vector.tensor_tensor(sb[:], wsqs[2][:], wsqs[3][:],
                                    op=ALU.add)
            st = wsqpool.tile([128, GROUP_COLS], F16, tag="wsqt")
            nc.vector.tensor_tensor(st[:], sa[:], sb[:], op=ALU.add)
            nrow = npool.tile([1, GROUP_COLS], F32, tag="nrow")
            s0 = 0
            for sw in (512, 512, 256):
                nq = nqpool.tile([1, 512], F32, tag="nq")
                nc.tensor.matmul(nq[:1, :sw], ones_h[:], st[:, s0:s0 + sw],
                                 start=True, stop=True)
                nc.scalar.copy(nrow[:, s0:s0 + sw], nq[:1, :sw])
                s0 += sw
            nc.sync.dma_start(nscr[:, g0:g0 + GROUP_COLS], nrow[:])
            ncol = npool.tile([128, TILES_PER_GROUP], F32, tag="ncol")
            nc.sync.dma_start(
                ncol[:],
                nscr[:, g0:g0 + GROUP_COLS].rearrange(
                    "a (t p) -> p (a t)", p=128))
            rg = _newton_rsqrt(nc, newtp, ncol[:], [128, TILES_PER_GROUP],
                               0.0447, 4, "nr")
            r64g = r64pool.tile([128, TILES_PER_GROUP], F32, tag="r64")
            nc.vector.tensor_scalar(r64g[:], rg[:], SCALE, None, op0=ALU.mult)
            wg_of[g] = (*wg_of[g], r64g)

        def main_group(g):
            wg, _, r64g = wg_of[g]
            t0 = g * TILES_PER_GROUP
            obuf = None
            ostart = 0
            for m in range(TILES_PER_GROUP):
                ti = t0 + m
                z = zpool.tile([128, N], F32, tag="z")
                for c in range(KCHUNKS):
                    nc.tensor.matmul(
                        z[:], wg[c][:, m * 128:(m + 1) * 128],
                        et[:, c * N:(c + 1) * N],
                        start=(c == 0), stop=(c == KCHUNKS - 1))
                ex = epool.tile([128, N], BF16, tag="ex")
                sm = spool.tile([128, 1], F32, tag="sm")
                nc.scalar.activation(ex[:], z[:], AFT.Exp,
                                     scale=r64g[:, m:m + 1],
                                     accum_out=sm[:])
                smf = spool.tile([128, 1], F32, tag="smf")
                nc.vector.tensor_tensor(smf[:], sm[:],
                                        dsm_all[:, ti:ti + 1], op=ALU.add)
                nc.vector.reciprocal(rp_all[:, ti:ti + 1], smf[:])
                o = opool.tile([128, N], BF16, tag="o")
                nc.vector.tensor_scalar(o[:], ex[:], rp_all[:, ti:ti + 1],
                                        None, op0=ALU.mult)
                out_dmas.append(
                    nc.sync.dma_start(out_t[:, ti, :], o[:]))
            del wg_of[g]

        for g in range(1, 3):
            load_and_square(g)
        norm_chain(0)
        norm_chain(1)
        for g in range(N_GROUPS):
            main_group(g)
            if g + 3 < N_GROUPS:
                load_and_square(g + 3)
            if g + 2 < N_GROUPS:
                norm_chain(g + 2)

        # ---------------- tail: patch target entries --------------------
        v_all = spool.tile([128, KCHUNKS], F32, tag="vall")
        for c in range(KCHUNKS):
            gq = gpool.tile([128, N_TILES], F32, tag="g")
            nc.tensor.matmul(gq[:], h1tc[:, c * 128:(c + 1) * 128],
                             rp_all[:], start=True, stop=True)
            g2 = spool.tile([128, N_TILES], F32, tag="g2")
            nc.vector.tensor_tensor(g2[:], gq[:],
                                    h2c[:, c * N_TILES:(c + 1) * N_TILES],
                                    op=ALU.mult)
            rpt = spool.tile([128, 1], F32, tag="rpt")
            nc.vector.reduce_sum(rpt[:], g2[:], axis=mybir.AxisListType.X)
            nc.vector.tensor_tensor(v_all[:, c:c + 1], en[:, c:c + 1],
                                    rpt[:], op=ALU.mult)
        nc.sync.dma_start(patch[:, :], v_all[:])

    nc.compile()
    return nc


_NC_CACHE = None


def _get_program():
    global _NC_CACHE
    if _NC_CACHE is None:
        _NC_CACHE = build_program()
    return _NC_CACHE


def _shard_inputs(embedding_batch, w_param, target_batch):
    emb = np.ascontiguousarray(embedding_batch, dtype=np.float32)
    wp = np.asarray(w_param, dtype=np.float32).reshape(D, C)
    tgt = np.asarray(target_batch, dtype=np.float32)

    embT = np.ascontiguousarray(emb.T.astype(np.float16))
    labels = np.argmax(tgt, axis=1).astype(np.int64)
    wt = np.ascontiguousarray(wp[:, labels].astype(np.float16))

    js = np.arange(N)
    in_maps = []
    for k in range(N_CORES):
        lo = k * C_SHARD
        in_shard = (labels >= lo) & (labels < lo + C_SHARD)
        lc = np.where(in_shard, labels - lo, 0)

        wk = np.ones((D, C_PAD), dtype=np.float16)
        wk[:, :C_SHARD] = wp[:, lo:lo + C_SHARD].astype(np.float16)
        # pre-block to [group, chunk, partition, cols] so every device load
        # is one contiguous 655KB DMA
        wk = np.ascontiguousarray(
            wk.reshape(KCHUNKS, 128, N_GROUPS, GROUP_COLS)
            .transpose(2, 0, 1, 3))

        h1 = np.zeros((N, 128), dtype=np.float32)
        h1[js[in_shard], lc[in_shard] % 128] = 1.0
        h2 = np.zeros((N, N_TILES), dtype=np.float32)
        h2[js[in_shard], lc[in_shard] // 128] = 1.0
        in_maps.append({
            "embT": embT, "w": wk, "wt": wt,
            "h1": h1, "h1t": np.ascontiguousarray(h1.T),
            "h2j": h2,
        })
    return in_maps


def run(inputs, trace=False):
    nc = _get_program()
    in_maps = _shard_inputs(**inputs)
    res = run_bass_kernel_spmd(nc, in_maps, core_ids=list(range(N_CORES)),
                               trace=trace)
    full = np.empty((N, C), dtype=np.float32)
    for k in range(N_CORES):
        full[:, k * C_SHARD:(k + 1) * C_SHARD] = \
            res.results[k]["out"][:C_SHARD, :].astype(np.float32).T
    # place the device-computed margin patch values at the target entries
    labels = np.argmax(np.asarray(inputs["target_batch"]), axis=1)
    js = np.arange(N)
    owner = labels // C_SHARD
    for k in range(N_CORES):
        sel = owner == k
        pk = np.asarray(res.results[k]["patch"], dtype=np.float32)
        full[js[sel], labels[sel]] = pk[js[sel] % 128, js[sel] // 128]
    return full, res


def kernel(embedding_batch, w_param, target_batch):
    full, _ = run(dict(embedding_batch=embedding_batch, w_param=w_param,
                       target_batch=target_batch))
    return full
